# revision 19
# baseline (speedup 1.0000x reference)
"""GQA attention kernel for 8 TRN2 NeuronCores (Bass/Tile, SPMD).

Sharding: core c -> (batch b = c // 4, kv-head kv = c % 4). Each core computes
the 4 query heads of its kv group for its batch and a partial (transposed)
output projection; the host sums the 4 partials per batch.

v2: all matmul operands are bf16 (psum accumulation stays fp32). In fp32 mode
the PE serializes a 128-cycle LDWEIGHTS before every matmul (no FWL, no
background weight buffer) and p-state churn keeps the array at ~1.2 GHz; bf16
streams at ~N/2.4 ns with weight loads hidden. Attention is computed in
transposed layout throughout:
  QT/KT [hd, t]  ->  S.T [k, q] psum  ->  exp on ACT  ->  P.T [k, q] bf16
  O.T [hd, q] = V[k, hd].T-accumulated PV matmuls
Query heads are processed in PAIRS sharing a [128, 2, 512] psum tile so one
ACT exp covers both heads (the 352-cycle ACT fixed cost halves). Softmax
denominators come from ones-vector matmuls (cheap LDWEIGHTS, balances the
ACT-bound exp stream); normalization is deferred: po is evicted unnormalized
(frees psum fast) and scaled in place ~2 pairs later once the gpsimd
partition-broadcast of 1/l lands. RoPE's rotate-half is a +-1 permutation
matmul on the hd (partition) axis.
"""

import os
import sys

import numpy as np

for _p in ("/opt/trn_rl_repo", "/root/.axon_site/_ro/trn_rl_repo"):
    if os.path.isdir(_p) and _p not in sys.path:
        sys.path.insert(0, _p)

import ml_dtypes  # noqa: E402

import concourse.bass as bass  # noqa: E402
import concourse.mybir as mybir  # noqa: E402
from concourse import bacc  # noqa: E402
from concourse.tile import TileContext  # noqa: E402
from concourse.bass_utils import run_bass_kernel_spmd  # noqa: E402

B, T, D = 2, 2048, 2048
H, HKV, HD = 16, 4, 128
G = H // HKV            # query heads per kv head (= per core)
EQ = G * HD             # 512: query-projection rows per core
P = 128
TC = 512                # t-chunk (free dim of most matmuls)
NJ = T // TC            # 4 chunks
DT = D // P             # 16 contraction tiles
SCALE = 1.0 / float(np.sqrt(HD))

F32 = mybir.dt.float32
BF16 = mybir.dt.bfloat16
EXP = mybir.ActivationFunctionType.Exp
BF16_NP = ml_dtypes.bfloat16

_CACHE = {}


def _build():
    nc = bacc.Bacc("TRN2", target_bir_lowering=False, debug=False)

    # All inputs arrive pre-transposed into SBUF layout (partition dim first,
    # contiguous per partition) so every DMA runs at full descriptor rate.
    xT = nc.declare_dram_parameter("xT", [P, NJ, 4, 4, TC], BF16, isOutput=False)
    wqT = nc.declare_dram_parameter("wqT", [P, DT, EQ], BF16, isOutput=False)
    wkT = nc.declare_dram_parameter("wkT", [P, DT, HD], BF16, isOutput=False)
    wvT = nc.declare_dram_parameter("wvT", [P, DT, HD], BF16, isOutput=False)
    woT = nc.declare_dram_parameter("woT", [P, G, D], BF16, isOutput=False)
    cosT = nc.declare_dram_parameter("cosT", [HD, T], BF16, isOutput=False)
    sinT = nc.declare_dram_parameter("sinT", [HD, T], BF16, isOutput=False)
    rmat = nc.declare_dram_parameter("rmat", [HD, HD], BF16, isOutput=False)
    iden = nc.declare_dram_parameter("iden", [P, P], F32, isOutput=False)
    masks = nc.declare_dram_parameter("masks", [P, 2, P], BF16, isOutput=False)
    ones_k = nc.declare_dram_parameter("ones_k", [P, 1], BF16, isOutput=False)
    yT = nc.declare_dram_parameter("yT", [D, T], F32, isOutput=True)

    with TileContext(nc) as tc:
        with (
            tc.tile_pool(name="const", bufs=1) as cst,
            tc.tile_pool(name="kv", bufs=1) as kvp,
            tc.tile_pool(name="ot", bufs=1) as otp,
            tc.tile_pool(name="wts", bufs=1) as wts,
            tc.tile_pool(name="xs", bufs=1) as xs,
        ):
            # Constants ride the gpsimd SWDGE ring so they don't delay the
            # weight/x loads on the sync HWDGE ring.
            cos_sb = cst.tile([HD, T], BF16, tag="cos")
            sin_sb = cst.tile([HD, T], BF16, tag="sin")
            rmat_sb = cst.tile([HD, HD], BF16, tag="rmat")
            iden_sb = cst.tile([P, P], F32, tag="iden")
            mask_sb = cst.tile([P, 2, P], BF16, tag="mask")
            onek_sb = cst.tile([P, 1], BF16, tag="onek")
            nc.gpsimd.dma_start(cos_sb[:], cosT[:])
            nc.gpsimd.dma_start(sin_sb[:], sinT[:])
            nc.gpsimd.dma_start(rmat_sb[:], rmat[:])
            nc.gpsimd.dma_start(iden_sb[:], iden[:])
            nc.gpsimd.dma_start(mask_sb[:], masks[:])
            nc.gpsimd.dma_start(onek_sb[:], ones_k[:])

            kt_sb = kvp.tile([HD, T], BF16, tag="kt")
            v_sb = kvp.tile([P, DT, HD], BF16, tag="v")
            otn = otp.tile([HD, G, T], BF16, tag="otn")

            # Input DMAs, all on the sync ring, emitted in consumption order.
            # First chain (V proj of j=0) needs wv + x(0,0) only, so those go
            # first; wo goes last (C phase starts ~180us in).
            wq_sb = wts.tile([P, DT, EQ], BF16, tag="wq")
            wk_sb = wts.tile([P, DT, HD], BF16, tag="wk")
            wv_sb = wts.tile([P, DT, HD], BF16, tag="wv")
            wo_sb = wts.tile([P, G, D], BF16, tag="wo")
            xq_sb = {}

            def load_x_quarter(j, q):
                t = xs.tile([P, 4, TC], BF16, tag=f"x{j}{q}", name=f"x{j}{q}")
                nc.sync.dma_start(t[:], xT[:, j, q])
                xq_sb[(j, q)] = t

            # Weights ride the scalar HWDGE ring (ACT is idle at boot), x
            # chunks the sync ring, wo the gpsimd ring behind the constants —
            # three rings transfer in parallel during startup.
            nc.scalar.dma_start(wv_sb[:], wvT[:])
            nc.scalar.dma_start(wk_sb[:], wkT[:])
            for q in range(4):
                nc.scalar.dma_start(wq_sb[:, 4 * q:4 * q + 4],
                                    wqT[:, 4 * q:4 * q + 4])
            for j in range(NJ):
                for q in range(4):
                    load_x_quarter(j, q)
            for g in range(G):
                nc.gpsimd.dma_start(wo_sb[:, g], woT[:, g])

            with (
                tc.tile_pool(name="qk", bufs=2) as qk,
                tc.tile_pool(name="work", bufs=4) as wk,
                tc.tile_pool(name="rtmp", bufs=2) as rtmp,
                tc.tile_pool(name="vt", bufs=2) as vtp,
                tc.tile_pool(name="small", bufs=2) as sml,
                tc.tile_pool(name="ps_s", bufs=2, space="PSUM") as ps_s,
                tc.tile_pool(name="ps_o", bufs=2, space="PSUM") as ps_o,
                tc.tile_pool(name="ps_lb", bufs=1, space="PSUM") as ps_lb,
            ):
                # Half-granular rotation over the shared [128, 2, 512] psum
                # tiles: A-phase chain accumulators, rope temporaries, and the
                # V-transpose staging all take halves; B-phase S tiles take
                # whole tiles from the same tag so the 4 banks are reused.
                _half = {"tile": None, "idx": 1}

                def half_tile():
                    # returns [P, TC] fp32 psum AP (one half of a pair tile)
                    if _half["idx"] == 1:
                        _half["tile"] = ps_s.tile([P, 2, TC], F32, tag="s",
                                                  name="sh")
                        _half["idx"] = 0
                        return _half["tile"][:, 0]
                    _half["idx"] = 1
                    return _half["tile"][:, 1]

                # Deferred per-head normalizations: (h, jsl, binv_half_ap).
                # Flushed >=2 pairs later so the gpsimd broadcast has landed;
                # otherwise the DVE FIFO would stall behind it.
                pending_norm = []

                def flush_norms(keep):
                    while len(pending_norm) > keep:
                        h, jsl_, binv_h = pending_norm.pop(0)
                        nc.vector.tensor_mul(out=otn[:, h, jsl_],
                                             in0=otn[:, h, jsl_], in1=binv_h)

                def finish_rope(s, t1, jsl):
                    # s <- s*cos + rotate_half(s)*sin; t1 = s*cos precomputed
                    pr = half_tile()
                    nc.tensor.matmul(pr, rmat_sb[:], s, start=True, stop=True)
                    nc.vector.tensor_mul(out=s, in0=pr, in1=sin_sb[:, jsl])
                    nc.vector.tensor_add(out=s, in0=s, in1=t1[:])

                # Software-pipelined schedule: B_j(pair0), A_{j+1} first
                # half, B_j(pair1), A_{j+1} second half. The A chains (pure
                # PE work, no ACT dependency) fill the PE bubbles B's
                # ACT-bound stretches create, and B's exps drain while the
                # PE runs A chains.
                qt_of, vt_of, rope_of = {}, {}, {}

                def emit_A(j, part):
                    jsl = slice(j * TC, (j + 1) * TC)
                    if part == 0:
                        qt_of[j] = qk.tile([HD, G, TC], BF16, tag="qt",
                                           name="qt")
                        # vt stays fp32: the PE transpose requires out dtype
                        # == in dtype, and the staging halves are fp32 psum.
                        vt_of[j] = vtp.tile([HD, TC], F32, tag="vt", name="vt")
                        rope_of[j] = []
                    qt, vt, rope_q = qt_of[j], vt_of[j], rope_of[j]
                    for a in (range(3) if part == 0 else range(3, 6)):
                        acc = half_tile()
                        for dt in range(DT):
                            if a == 0:
                                lhsT = wv_sb[:, dt]
                            elif a == 1:
                                lhsT = wk_sb[:, dt]
                            else:
                                h = a - 2
                                lhsT = wq_sb[:, dt, h * HD:(h + 1) * HD]
                            nc.tensor.matmul(acc, lhsT,
                                             xq_sb[(j, dt // 4)][:, dt % 4],
                                             start=(dt == 0), stop=(dt == DT - 1))
                        if a == 0:
                            nc.vector.tensor_copy(vt[:], acc)
                        else:
                            s = kt_sb[:, jsl] if a == 1 else qt[:, a - 2]
                            nc.vector.tensor_copy(s, acc)
                            t1 = rtmp.tile([HD, TC], BF16, tag="t1")
                            nc.vector.tensor_mul(out=t1[:], in0=s,
                                                 in1=cos_sb[:, jsl])
                            rope_q.append((s, t1))
                        if a == 2:
                            # V transpose: 4 PE transposes into quarters of a
                            # staging half, one DVE copy into v_sb.
                            tp = half_tile()
                            for tt in range(NJ):
                                nc.tensor.transpose(
                                    tp[:, tt * P:(tt + 1) * P],
                                    vt[:, tt * P:(tt + 1) * P], iden_sb[:])
                            nc.vector.tensor_copy(v_sb[:, NJ * j:NJ * j + 4],
                                                  tp)
                        if len(rope_q) >= 3:
                            finish_rope(*rope_q.pop(0), jsl)
                    if part == 1:
                        while rope_q:
                            finish_rope(*rope_q.pop(0), jsl)

                def emit_B_pair(j, hp):
                    jsl = slice(j * TC, (j + 1) * TC)
                    qt = qt_of[j]
                    nk = 4 * (j + 1)
                    DEPTH = 2  # exp/mask run two S-pair-tiles ahead of PV/l
                    h0, h1 = 2 * hp, 2 * hp + 1
                    flush_norms(2)
                    po = {hh: ps_o.tile([P, TC], F32, tag="o", name="po")
                          for hh in (h0, h1)}
                    pl = ps_lb.tile([1, 2, TC], F32, tag="lb", name="pl")
                    pipe = []

                    def drain():
                        ppt, pkt, qs = pipe.pop(0)
                        st = dict(start=(pkt == 0), stop=(pkt == nk - 1))
                        nc.tensor.matmul(pl[:, 0, qs], onek_sb[:],
                                         ppt[:, 0, qs], **st)
                        nc.tensor.matmul(pl[:, 1, qs], onek_sb[:],
                                         ppt[:, 1, qs], **st)
                        nc.tensor.matmul(po[h0][:, qs], v_sb[:, pkt],
                                         ppt[:, 0, qs], **st)
                        nc.tensor.matmul(po[h1][:, qs], v_sb[:, pkt],
                                         ppt[:, 1, qs], **st)
                        if pkt == nk - 1:
                            # Evict unnormalized (frees po psum fast), then
                            # defer the normalize until the 1/l broadcast is
                            # done.
                            nc.vector.tensor_copy(otn[:, h0, jsl], po[h0][:])
                            nc.vector.tensor_copy(otn[:, h1, jsl], po[h1][:])
                            rinv = sml.tile([1, 2, TC], F32, tag="rinv")
                            nc.vector.reciprocal_approx_fast(rinv[:], pl[:])
                            rinv_b = sml.tile([1, 2, TC], BF16, tag="rinvb")
                            nc.vector.tensor_copy(rinv_b[:], rinv[:])
                            binv = sml.tile([P, 2, TC], BF16, tag="binv")
                            nc.gpsimd.partition_broadcast(binv[:], rinv_b[:])
                            pending_norm.append((h0, jsl, binv[:, 0]))
                            pending_norm.append((h1, jsl, binv[:, 1]))

                    for kt in range(nk):
                        m = kt - 4 * j
                        off = 0 if m < 0 else P * m
                        qs = slice(off, TC)
                        pss = ps_s.tile([P, 2, TC], F32, tag="s", name="pss")
                        nc.tensor.matmul(pss[:, 0, qs],
                                         kt_sb[:, kt * P:(kt + 1) * P],
                                         qt[:, h0, qs], start=True, stop=True)
                        nc.tensor.matmul(pss[:, 1, qs],
                                         kt_sb[:, kt * P:(kt + 1) * P],
                                         qt[:, h1, qs], start=True, stop=True)
                        pt = wk.tile([P, 2, TC], BF16, tag="pt")
                        nc.scalar.activation(pt[:, :, qs], pss[:, :, qs],
                                             EXP, scale=SCALE)
                        if m >= 0:
                            ssl = slice(off, off + P)
                            nc.vector.tensor_mul(out=pt[:, :, ssl],
                                                 in0=pt[:, :, ssl],
                                                 in1=mask_sb[:])
                        pipe.append((pt, kt, qs))
                        if len(pipe) > DEPTH:
                            drain()
                    while pipe:
                        drain()

                emit_A(0, 0)
                emit_A(0, 1)
                for j in range(NJ):
                    emit_B_pair(j, 0)
                    if j + 1 < NJ:
                        emit_A(j + 1, 0)
                    emit_B_pair(j, 1)
                    if j + 1 < NJ:
                        emit_A(j + 1, 1)
                flush_norms(0)

                # ---- C: output projection, yT = woT.T @ otn (transposed
                # partial). Runs on the same psum pool (no pool-closure
                # barrier); a [128, 2, 512] tile serves two tj chunks so one
                # ACT eviction + one 0.5 MiB DMA cover 8 matmuls (keeps C
                # PE-bound and the final tail short).
                with tc.tile_pool(name="yout", bufs=3) as yop:
                    for dt in range(DT):
                        for tjp in range(NJ // 2):
                            py = ps_s.tile([P, 2, TC], F32, tag="s", name="py")
                            for hh in range(2):
                                tj = 2 * tjp + hh
                                tsl = slice(tj * TC, (tj + 1) * TC)
                                for g in range(G):
                                    nc.tensor.matmul(
                                        py[:, hh],
                                        wo_sb[:, g, dt * P:(dt + 1) * P],
                                        otn[:, g, tsl],
                                        start=(g == 0), stop=(g == G - 1))
                            y_sb = yop.tile([P, 2 * TC], F32, tag="ysb")
                            nc.scalar.copy(y_sb[:], py[:])
                            nc.sync.dma_start(
                                yT[dt * P:(dt + 1) * P,
                                   2 * tjp * TC:2 * (tjp + 1) * TC],
                                y_sb[:])

    nc.compile()
    return nc


def _host_shards(inputs):
    x = np.asarray(inputs["x"], dtype=np.float32)
    cos = np.asarray(inputs["cos"], dtype=np.float32)
    sin = np.asarray(inputs["sin"], dtype=np.float32)
    Wq = np.asarray(inputs["Wq"], dtype=np.float32)
    Wk = np.asarray(inputs["Wk"], dtype=np.float32)
    Wv = np.asarray(inputs["Wv"], dtype=np.float32)
    Wo = np.asarray(inputs["Wo"], dtype=np.float32)

    def bf(a):
        return np.ascontiguousarray(a.astype(BF16_NP))

    cosT = bf(cos.T)
    sinT = bf(sin.T)
    rmat_ = np.zeros((HD, HD), np.float32)
    hf = HD // 2
    for i in range(hf):
        rmat_[i + hf, i] = -1.0     # out[m<64] = -q[m+64]
        rmat_[i, i + hf] = 1.0      # out[m>=64] = q[m-64]
    rmat_ = bf(rmat_)
    iden_ = np.eye(P, dtype=np.float32)
    kk = np.arange(P)[:, None, None]
    qq = np.arange(P)[None, None, :]
    masks_ = bf(np.broadcast_to((qq >= kk), (P, 2, P)).astype(np.float32))
    ones_ = bf(np.ones((P, 1), np.float32))

    def to_sbuf_layout(wT, cols):
        # [D_contract, cols] -> [P, D_contract//P, cols], partition dim first
        return bf(wT.reshape(-1, P, cols).transpose(1, 0, 2))

    # x[b].T is [d, t]; device layout [p, j, q, dtq, t'] with d = (4q+dtq)*P+p
    # and t = j*TC + t' makes each (j, q) quarter-load fully contiguous.
    xTs = [bf(x[b].T.reshape(4, 4, P, NJ, TC).transpose(2, 3, 0, 1, 4))
           for b in range(B)]
    wqTs = [to_sbuf_layout(Wq[kv * EQ:(kv + 1) * EQ].T, EQ) for kv in range(HKV)]
    wkTs = [to_sbuf_layout(Wk[kv * HD:(kv + 1) * HD].T, HD) for kv in range(HKV)]
    wvTs = [to_sbuf_layout(Wv[kv * HD:(kv + 1) * HD].T, HD) for kv in range(HKV)]
    woTs = [to_sbuf_layout(Wo[:, kv * EQ:(kv + 1) * EQ].T, D) for kv in range(HKV)]

    in_maps = []
    for c in range(8):
        b, kv = divmod(c, HKV)
        in_maps.append({
            "xT": xTs[b], "wqT": wqTs[kv], "wkT": wkTs[kv], "wvT": wvTs[kv],
            "woT": woTs[kv], "cosT": cosT, "sinT": sinT, "rmat": rmat_,
            "iden": iden_, "masks": masks_, "ones_k": ones_,
        })
    return in_maps


def get_nc():
    if "nc" not in _CACHE:
        _CACHE["nc"] = _build()
    return _CACHE["nc"]


def run(inputs, **kw):
    nc = get_nc()
    in_maps = _host_shards(inputs)
    res = run_bass_kernel_spmd(nc, in_maps, core_ids=list(range(8)), **kw)
    out = np.zeros((B, T, D), np.float32)
    for c in range(8):
        b = c // HKV
        out[b] += res.results[c]["yT"].T
    return out, res


def kernel(**inputs) -> np.ndarray:
    out, _ = run(inputs)
    return out


# revision 22
# speedup vs baseline: 1.1550x; 1.1550x over previous
"""GQA attention kernel for 8 TRN2 NeuronCores (Bass/Tile, SPMD).

Sharding: core c -> (batch b = c // 4, kv-head kv = c % 4). Each core computes
the 4 query heads of its kv group for its batch and a partial (transposed)
output projection; the host sums the 4 partials per batch.

v2: all matmul operands are bf16 (psum accumulation stays fp32). In fp32 mode
the PE serializes a 128-cycle LDWEIGHTS before every matmul (no FWL, no
background weight buffer) and p-state churn keeps the array at ~1.2 GHz; bf16
streams at ~N/2.4 ns with weight loads hidden. Attention is computed in
transposed layout throughout:
  QT/KT [hd, t]  ->  S.T [k, q] psum  ->  exp on ACT  ->  P.T [k, q] bf16
  O.T [hd, q] = V[k, hd].T-accumulated PV matmuls
Query heads are processed in PAIRS sharing a [128, 2, 512] psum tile so one
ACT exp covers both heads (the 352-cycle ACT fixed cost halves). Softmax
denominators come from ones-vector matmuls (cheap LDWEIGHTS, balances the
ACT-bound exp stream); normalization is deferred: po is evicted unnormalized
(frees psum fast) and scaled in place ~2 pairs later once the gpsimd
partition-broadcast of 1/l lands. RoPE's rotate-half is a +-1 permutation
matmul on the hd (partition) axis.
"""

import os
import sys

import numpy as np

for _p in ("/opt/trn_rl_repo", "/root/.axon_site/_ro/trn_rl_repo"):
    if os.path.isdir(_p) and _p not in sys.path:
        sys.path.insert(0, _p)

import ml_dtypes  # noqa: E402

import concourse.bass as bass  # noqa: E402
import concourse.mybir as mybir  # noqa: E402
from concourse import bacc  # noqa: E402
from concourse.tile import TileContext  # noqa: E402
from concourse.bass_utils import run_bass_kernel_spmd  # noqa: E402

B, T, D = 2, 2048, 2048
H, HKV, HD = 16, 4, 128
G = H // HKV            # query heads per kv head (= per core)
EQ = G * HD             # 512: query-projection rows per core
P = 128
TC = 512                # t-chunk (free dim of most matmuls)
NJ = T // TC            # 4 chunks
DT = D // P             # 16 contraction tiles
SCALE = 1.0 / float(np.sqrt(HD))

F32 = mybir.dt.float32
BF16 = mybir.dt.bfloat16
EXP = mybir.ActivationFunctionType.Exp
BF16_NP = ml_dtypes.bfloat16

_CACHE = {}


def _build():
    nc = bacc.Bacc("TRN2", target_bir_lowering=False, debug=False)

    # All inputs arrive pre-transposed into SBUF layout (partition dim first,
    # contiguous per partition) so every DMA runs at full descriptor rate.
    xT = nc.declare_dram_parameter("xT", [P, NJ, 4, 4, TC], BF16, isOutput=False)
    wqT = nc.declare_dram_parameter("wqT", [P, DT, EQ], BF16, isOutput=False)
    wkT = nc.declare_dram_parameter("wkT", [P, DT, HD], BF16, isOutput=False)
    wvT = nc.declare_dram_parameter("wvT", [P, DT, HD], BF16, isOutput=False)
    woT = nc.declare_dram_parameter("woT", [P, G, D], BF16, isOutput=False)
    cosT = nc.declare_dram_parameter("cosT", [HD, T], BF16, isOutput=False)
    sinT = nc.declare_dram_parameter("sinT", [HD, T], BF16, isOutput=False)
    rmat = nc.declare_dram_parameter("rmat", [HD, HD], BF16, isOutput=False)
    iden = nc.declare_dram_parameter("iden", [P, P], F32, isOutput=False)
    masks = nc.declare_dram_parameter("masks", [P, 2, P], BF16, isOutput=False)
    ones_k = nc.declare_dram_parameter("ones_k", [P, 1], BF16, isOutput=False)
    yT = nc.declare_dram_parameter("yT", [D, T], F32, isOutput=True)

    with TileContext(nc) as tc:
        with (
            tc.tile_pool(name="const", bufs=1) as cst,
            tc.tile_pool(name="kv", bufs=1) as kvp,
            tc.tile_pool(name="ot", bufs=1) as otp,
            tc.tile_pool(name="wts", bufs=1) as wts,
            tc.tile_pool(name="xs", bufs=1) as xs,
        ):
            # Constants ride the gpsimd SWDGE ring so they don't delay the
            # weight/x loads on the sync HWDGE ring.
            cos_sb = cst.tile([HD, T], BF16, tag="cos")
            sin_sb = cst.tile([HD, T], BF16, tag="sin")
            rmat_sb = cst.tile([HD, HD], BF16, tag="rmat")
            iden_sb = cst.tile([P, P], F32, tag="iden")
            mask_sb = cst.tile([P, 2, P], BF16, tag="mask")
            onek_sb = cst.tile([P, 1], BF16, tag="onek")
            nc.gpsimd.dma_start(cos_sb[:], cosT[:])
            nc.gpsimd.dma_start(sin_sb[:], sinT[:])
            nc.gpsimd.dma_start(rmat_sb[:], rmat[:])
            nc.gpsimd.dma_start(iden_sb[:], iden[:])
            nc.gpsimd.dma_start(mask_sb[:], masks[:])
            nc.gpsimd.dma_start(onek_sb[:], ones_k[:])

            kt_sb = kvp.tile([HD, T], BF16, tag="kt")
            v_sb = kvp.tile([P, DT, HD], BF16, tag="v")
            otn = otp.tile([HD, G, T], BF16, tag="otn")

            # Input DMAs, all on the sync ring, emitted in consumption order.
            # First chain (V proj of j=0) needs wv + x(0,0) only, so those go
            # first; wo goes last (C phase starts ~180us in).
            wq_sb = wts.tile([P, DT, EQ], BF16, tag="wq")
            wk_sb = wts.tile([P, DT, HD], BF16, tag="wk")
            wv_sb = wts.tile([P, DT, HD], BF16, tag="wv")
            wo_sb = wts.tile([P, G, D], BF16, tag="wo")
            xq_sb = {}

            def load_x_quarter(j, q):
                t = xs.tile([P, 4, TC], BF16, tag=f"x{j}{q}", name=f"x{j}{q}")
                nc.sync.dma_start(t[:], xT[:, j, q])
                xq_sb[(j, q)] = t

            # Weights ride the scalar HWDGE ring (ACT is idle at boot), x
            # chunks the sync ring, wo the gpsimd ring behind the constants —
            # three rings transfer in parallel during startup.
            nc.scalar.dma_start(wv_sb[:], wvT[:])
            nc.scalar.dma_start(wk_sb[:], wkT[:])
            for q in range(4):
                nc.scalar.dma_start(wq_sb[:, 4 * q:4 * q + 4],
                                    wqT[:, 4 * q:4 * q + 4])
            for j in range(NJ):
                for q in range(4):
                    load_x_quarter(j, q)
            for g in range(G):
                nc.gpsimd.dma_start(wo_sb[:, g], woT[:, g])

            with (
                tc.tile_pool(name="qk", bufs=2) as qk,
                tc.tile_pool(name="work", bufs=5) as wk,
                tc.tile_pool(name="rtmp", bufs=2) as rtmp,
                tc.tile_pool(name="vt", bufs=2) as vtp,
                tc.tile_pool(name="small", bufs=2) as sml,
                tc.tile_pool(name="ps_s", bufs=2, space="PSUM") as ps_s,
                tc.tile_pool(name="ps_o", bufs=2, space="PSUM") as ps_o,
                tc.tile_pool(name="ps_lb", bufs=1, space="PSUM") as ps_lb,
            ):
                # Half-granular rotation over the shared [128, 2, 512] psum
                # tiles: A-phase chain accumulators, rope temporaries, and the
                # V-transpose staging all take halves; B-phase S tiles take
                # whole tiles from the same tag so the 4 banks are reused.
                _half = {"tile": None, "idx": 1}

                def half_tile():
                    # returns [P, TC] fp32 psum AP (one half of a pair tile)
                    if _half["idx"] == 1:
                        _half["tile"] = ps_s.tile([P, 2, TC], F32, tag="s",
                                                  name="sh")
                        _half["idx"] = 0
                        return _half["tile"][:, 0]
                    _half["idx"] = 1
                    return _half["tile"][:, 1]

                # Deferred per-head normalizations: (h, jsl, binv_half_ap).
                # Flushed >=2 pairs later so the gpsimd broadcast has landed;
                # otherwise the DVE FIFO would stall behind it.
                pending_norm = []

                def flush_norms(keep):
                    while len(pending_norm) > keep:
                        h, jsl_, binv_h = pending_norm.pop(0)
                        nc.vector.tensor_mul(out=otn[:, h, jsl_],
                                             in0=otn[:, h, jsl_], in1=binv_h)

                def finish_rope(s, t1, jsl):
                    # s <- s*cos + rotate_half(s)*sin; t1 = s*cos precomputed
                    pr = half_tile()
                    nc.tensor.matmul(pr, rmat_sb[:], s, start=True, stop=True)
                    nc.vector.tensor_mul(out=s, in0=pr, in1=sin_sb[:, jsl])
                    nc.vector.tensor_add(out=s, in0=s, in1=t1[:])

                for j in range(NJ):
                    jsl = slice(j * TC, (j + 1) * TC)
                    # ---- A_j: projections of t-chunk j + RoPE + V transpose.
                    # Chain order V,K,Q0..Q3; each chain's RoPE is emitted one
                    # chain later so its eviction + cos-mul hide under matmuls.
                    qt = qk.tile([HD, G, TC], BF16, tag="qt")
                    # vt stays fp32: the PE transpose requires out dtype ==
                    # in dtype, and the staging halves are fp32 psum.
                    vt = vtp.tile([HD, TC], F32, tag="vt")
                    rope_q = []
                    for a in range(6):
                        acc = half_tile()
                        for dt in range(DT):
                            if a == 0:
                                lhsT = wv_sb[:, dt]
                            elif a == 1:
                                lhsT = wk_sb[:, dt]
                            else:
                                h = a - 2
                                lhsT = wq_sb[:, dt, h * HD:(h + 1) * HD]
                            nc.tensor.matmul(acc, lhsT,
                                             xq_sb[(j, dt // 4)][:, dt % 4],
                                             start=(dt == 0), stop=(dt == DT - 1))
                        if a == 0:
                            nc.vector.tensor_copy(vt[:], acc)
                        else:
                            s = kt_sb[:, jsl] if a == 1 else qt[:, a - 2]
                            nc.vector.tensor_copy(s, acc)
                            t1 = rtmp.tile([HD, TC], BF16, tag="t1")
                            nc.vector.tensor_mul(out=t1[:], in0=s,
                                                 in1=cos_sb[:, jsl])
                            rope_q.append((s, t1))
                        if a == 2:
                            # V transpose: 4 PE transposes into quarters of a
                            # staging half, one DVE copy into v_sb.
                            tp = half_tile()
                            for tt in range(NJ):
                                nc.tensor.transpose(
                                    tp[:, tt * P:(tt + 1) * P],
                                    vt[:, tt * P:(tt + 1) * P], iden_sb[:])
                            nc.vector.tensor_copy(v_sb[:, NJ * j:NJ * j + 4],
                                                  tp)
                        if len(rope_q) >= 3:
                            finish_rope(*rope_q.pop(0), jsl)
                    while rope_q:
                        finish_rope(*rope_q.pop(0), jsl)

                    # ---- B_j: attention for q-block j, heads in pairs.
                    # Diagonal k-tiles (m = kt-4j >= 0) only compute columns
                    # >= 128m; the masked triangle within the leading 128
                    # columns is zeroed on DVE after the exp.
                    nk = 4 * (j + 1)
                    DEPTH = 3  # exp/mask run three S-pair-tiles ahead of PV/l

                    for hp in range(2):
                        h0, h1 = 2 * hp, 2 * hp + 1
                        flush_norms(2)
                        po = {hh: ps_o.tile([P, TC], F32, tag="o", name="po")
                              for hh in (h0, h1)}
                        pl = ps_lb.tile([1, 2, TC], F32, tag="lb", name="pl")
                        pipe = []

                        def drain():
                            ppt, pkt, qs = pipe.pop(0)
                            st = dict(start=(pkt == 0), stop=(pkt == nk - 1))
                            nc.tensor.matmul(pl[:, 0, qs], onek_sb[:],
                                             ppt[:, 0, qs], **st)
                            nc.tensor.matmul(pl[:, 1, qs], onek_sb[:],
                                             ppt[:, 1, qs], **st)
                            nc.tensor.matmul(po[h0][:, qs], v_sb[:, pkt],
                                             ppt[:, 0, qs], **st)
                            nc.tensor.matmul(po[h1][:, qs], v_sb[:, pkt],
                                             ppt[:, 1, qs], **st)
                            if pkt == nk - 1:
                                # Evict unnormalized (frees po psum fast),
                                # then defer the normalize until the 1/l
                                # broadcast is done.
                                nc.vector.tensor_copy(otn[:, h0, jsl], po[h0][:])
                                nc.vector.tensor_copy(otn[:, h1, jsl], po[h1][:])
                                rinv = sml.tile([1, 2, TC], F32, tag="rinv")
                                nc.vector.reciprocal_approx_fast(rinv[:], pl[:])
                                rinv_b = sml.tile([1, 2, TC], BF16, tag="rinvb")
                                nc.vector.tensor_copy(rinv_b[:], rinv[:])
                                binv = sml.tile([P, 2, TC], BF16, tag="binv")
                                nc.gpsimd.partition_broadcast(binv[:], rinv_b[:])
                                pending_norm.append((h0, jsl, binv[:, 0]))
                                pending_norm.append((h1, jsl, binv[:, 1]))

                        for kt in range(nk):
                            m = kt - 4 * j
                            off = 0 if m < 0 else P * m
                            qs = slice(off, TC)
                            pss = ps_s.tile([P, 2, TC], F32, tag="s", name="pss")
                            nc.tensor.matmul(pss[:, 0, qs],
                                             kt_sb[:, kt * P:(kt + 1) * P],
                                             qt[:, h0, qs], start=True, stop=True)
                            nc.tensor.matmul(pss[:, 1, qs],
                                             kt_sb[:, kt * P:(kt + 1) * P],
                                             qt[:, h1, qs], start=True, stop=True)
                            pt = wk.tile([P, 2, TC], BF16, tag="pt")
                            nc.scalar.activation(pt[:, :, qs], pss[:, :, qs],
                                                 EXP, scale=SCALE)
                            if m >= 0:
                                ssl = slice(off, off + P)
                                nc.vector.tensor_mul(out=pt[:, :, ssl],
                                                     in0=pt[:, :, ssl],
                                                     in1=mask_sb[:])
                            pipe.append((pt, kt, qs))
                            if len(pipe) > DEPTH:
                                drain()
                        while pipe:
                            drain()
                flush_norms(0)

                # ---- C: output projection, yT = woT.T @ otn (transposed
                # partial). Runs on the same psum pool (no pool-closure
                # barrier); a [128, 2, 512] tile serves two tj chunks so one
                # ACT eviction + one 0.5 MiB DMA cover 8 matmuls (keeps C
                # PE-bound and the final tail short).
                with tc.tile_pool(name="yout", bufs=3) as yop:
                    for dt in range(DT):
                        for tjp in range(NJ // 2):
                            py = ps_s.tile([P, 2, TC], F32, tag="s", name="py")
                            for hh in range(2):
                                tj = 2 * tjp + hh
                                tsl = slice(tj * TC, (tj + 1) * TC)
                                for g in range(G):
                                    nc.tensor.matmul(
                                        py[:, hh],
                                        wo_sb[:, g, dt * P:(dt + 1) * P],
                                        otn[:, g, tsl],
                                        start=(g == 0), stop=(g == G - 1))
                            y_sb = yop.tile([P, 2 * TC], F32, tag="ysb")
                            nc.scalar.copy(y_sb[:], py[:])
                            nc.sync.dma_start(
                                yT[dt * P:(dt + 1) * P,
                                   2 * tjp * TC:2 * (tjp + 1) * TC],
                                y_sb[:])

    nc.compile()
    return nc


def _host_shards(inputs):
    x = np.asarray(inputs["x"], dtype=np.float32)
    cos = np.asarray(inputs["cos"], dtype=np.float32)
    sin = np.asarray(inputs["sin"], dtype=np.float32)
    Wq = np.asarray(inputs["Wq"], dtype=np.float32)
    Wk = np.asarray(inputs["Wk"], dtype=np.float32)
    Wv = np.asarray(inputs["Wv"], dtype=np.float32)
    Wo = np.asarray(inputs["Wo"], dtype=np.float32)

    def bf(a):
        return np.ascontiguousarray(a.astype(BF16_NP))

    cosT = bf(cos.T)
    sinT = bf(sin.T)
    rmat_ = np.zeros((HD, HD), np.float32)
    hf = HD // 2
    for i in range(hf):
        rmat_[i + hf, i] = -1.0     # out[m<64] = -q[m+64]
        rmat_[i, i + hf] = 1.0      # out[m>=64] = q[m-64]
    rmat_ = bf(rmat_)
    iden_ = np.eye(P, dtype=np.float32)
    kk = np.arange(P)[:, None, None]
    qq = np.arange(P)[None, None, :]
    masks_ = bf(np.broadcast_to((qq >= kk), (P, 2, P)).astype(np.float32))
    ones_ = bf(np.ones((P, 1), np.float32))

    def to_sbuf_layout(wT, cols):
        # [D_contract, cols] -> [P, D_contract//P, cols], partition dim first
        return bf(wT.reshape(-1, P, cols).transpose(1, 0, 2))

    # x[b].T is [d, t]; device layout [p, j, q, dtq, t'] with d = (4q+dtq)*P+p
    # and t = j*TC + t' makes each (j, q) quarter-load fully contiguous.
    xTs = [bf(x[b].T.reshape(4, 4, P, NJ, TC).transpose(2, 3, 0, 1, 4))
           for b in range(B)]
    wqTs = [to_sbuf_layout(Wq[kv * EQ:(kv + 1) * EQ].T, EQ) for kv in range(HKV)]
    wkTs = [to_sbuf_layout(Wk[kv * HD:(kv + 1) * HD].T, HD) for kv in range(HKV)]
    wvTs = [to_sbuf_layout(Wv[kv * HD:(kv + 1) * HD].T, HD) for kv in range(HKV)]
    woTs = [to_sbuf_layout(Wo[:, kv * EQ:(kv + 1) * EQ].T, D) for kv in range(HKV)]

    in_maps = []
    for c in range(8):
        b, kv = divmod(c, HKV)
        in_maps.append({
            "xT": xTs[b], "wqT": wqTs[kv], "wkT": wkTs[kv], "wvT": wvTs[kv],
            "woT": woTs[kv], "cosT": cosT, "sinT": sinT, "rmat": rmat_,
            "iden": iden_, "masks": masks_, "ones_k": ones_,
        })
    return in_maps


def get_nc():
    if "nc" not in _CACHE:
        _CACHE["nc"] = _build()
    return _CACHE["nc"]


def run(inputs, **kw):
    nc = get_nc()
    in_maps = _host_shards(inputs)
    res = run_bass_kernel_spmd(nc, in_maps, core_ids=list(range(8)), **kw)
    out = np.zeros((B, T, D), np.float32)
    for c in range(8):
        b = c // HKV
        out[b] += res.results[c]["yT"].T
    return out, res


def kernel(**inputs) -> np.ndarray:
    out, _ = run(inputs)
    return out
